# revision 27
# baseline (speedup 1.0000x reference)
"""GAT (2-layer, PyG-style) on 8 Trainium2 NeuronCores.

Approach
--------
Layer 1 has in_channels=1, so h = x @ W1 is rank-1: every per-edge quantity
reduces to scalars per node.  With s1[h] = sum_c W1[h,c]*att_src1[h,c] and
d1[h] = sum_c W1[h,c]*att_dst1[h,c]:

    e[i,h]   = leaky_relu(s1[h]*x[src_i] + d1[h]*x[dst_i])
    denom[d,h] = sum_{i->d} exp(e[i,h])          (max-shift skipped: |e| < ~10)
    z[d,h]     = sum_{i->d} exp(e[i,h]) * x[src_i]
    out1[d,h,c] = elu(W1[h,c]*z/(denom+eps) + b1[h,c])

Layer 2 (heads=1, out=1) similarly only needs the scalar h2 = out1 @ W2.

Sharding: dst-owner node sharding (12500 nodes/core).  Per core, nodes are
degree-sorted and packed into 98 blocks of 128 (partition dim); each node's
incoming edges occupy W columns (block-group-padded).  x[dst] is then a free
per-partition broadcast and segment sums are row reductions; only x[src]
needs a real gather.

The per-edge scalar gather is split across two engine paths that run
concurrently (POOL_COLS tunes the balance):
  - SWDGE path: per-column indirect DMAs (128 per-partition descriptors per
    instruction; one instruction can only carry one offset per partition on
    this walrus, and ~1us/instruction is the floor).
  - PE path: the table lives in SBUF as [256, 392] float32r; per slot
    column, the 128 q-values are broadcast across partitions with a one-hot
    row-select matmul, turned into two [128, 128] one-hot chunks with
    is_equal vs a partition iota (batched 4 columns per DVE op), row-gathered
    with two chained f32r matmuls into PSUM, and the lane is selected with a
    fused is_equal*mult (DVE) + accumulate-reduce on the Activation engine
    (accum_out).  f32r rounds the table to ~11 mantissa bits and the select
    products go through f16, final rel err ~9e-3 (tolerance 2e-2).

h2 is exchanged between layers with an AllGather of the per-core slices;
both layers use the same [256, 392] table shape (8*12544 = 100352 = 256*392).
Baseline TimelineSim 9.09 ms (all-SWDGE, 1 DMA sem) -> 1.84 ms here
(NUM_SWDGE_GLOBAL_SEMS=8 + hybrid gather).  (The dedicated
DMA-gather/ap-gather custom instructions do not compile on this toolchain
-- "ISA wrong length" even for load_library -- and walrus only encodes ~2
sem waits per instruction, hence the BIR post-pass below that splits excess
waits onto injected NoOps.)
"""
import sys
sys.path.insert(0, "/opt/trn_rl_repo")
import re
import numpy as np
import concourse.bass as bass
import concourse.mybir as mybir
import concourse.tile as tile
from concourse import library_config
from concourse.bass_utils import run_bass_kernel_spmd
from bass_rust import ScopedClock, VectorClock

N = 100000
NCORES = 8
NPC = N // NCORES          # nodes per core
P = 128
NBLK = (NPC + P - 1) // P  # 98
ROWS = NBLK * P            # 12544
NEG_SLOPE = 0.2
EPS = 1e-16
BIG_NEG = -1.0e30
USE_ACT_LRELU = False
GCHUNK = 96                # gather chunk, in slot-columns
TABL = 392                 # PE-gather table lane count ([256, 392] = 100352)
POOL_COLS = 760            # columns kept on the SWDGE indirect-DMA path

F32 = mybir.dt.float32
F32R = mybir.dt.float32r
F16 = mybir.dt.float16
I16 = mybir.dt.int16
I32 = mybir.dt.int32
AT = mybir.AluOpType
AF = mybir.ActivationFunctionType


# ---------------------------------------------------------------------------
# Tile tail-drain workaround: walrus TPB_CTRL codegen rejects a Drain with
# more than two sem waits; emit one NOP-wait per proc first.
def _split_drain_and_barrier(self, tick_clock, wait_clock):
    gc = tick_clock.global_clock
    ticks = [int(x) for x in re.findall(r"\d+", repr(gc))]
    for i, t in enumerate(ticks):
        if t <= 0:
            continue
        sub = VectorClock()
        sub.require_at_least(i, t)
        inst = self.nc.sync.nop()
        wait_clock.add_sem_waits(inst.ins, ScopedClock({None: sub}))
    self.nc.sync.drain()
    self.nc.all_engine_barrier()
    popped = self.nc._tile_sem_poison_stack.pop()
    assert popped is self._sem_poison
    self.nc.clear_and_free_semaphores(list(self.sems.allocated().values()))
    self.nc.all_engine_barrier()


tile.TileContext._drain_and_barrier = _split_drain_and_barrier

import concourse.tile_sem_assignment as _tsa
_tsa.NUM_SWDGE_GLOBAL_SEMS = 8
_tsa.NUM_HWDGE_SEMS = 8

# Walrus encodes at most ~2 sem waits per instruction; split any excess onto
# injected same-engine NoOps in the BIR JSON right before compilation.
import json as _json
from concourse import bass2jax as _b2j


def _split_waits_json(bir, max_keep=1):
    d = _json.loads(bir)
    ctr = [0]

    def fix_block(blk):
        out = []
        for inst in blk.get("instructions", []):
            si = inst.get("sync_info")
            waits = (si or {}).get("on_wait") or []
            if len(waits) > max_keep and inst.get("opcode") != "NoOp":
                keep = waits[-max_keep:]
                for w in waits[:-max_keep]:
                    ctr[0] += 1
                    out.append({"debug": inst.get("debug", 0),
                                "engine": inst["engine"], "ins": [],
                                "outs": [], "name": f"I-wsp{ctr[0]}",
                                "opcode": "NoOp",
                                "sync_info": {"on_update": [], "on_wait": [w]}})
                si["on_wait"] = keep
            out.append(inst)
        blk["instructions"] = out
        for sb in blk.get("blocks", []):
            fix_block(sb)

    for fn in d["functions"]:
        for blk in fn["blocks"]:
            fix_block(blk)
    return _json.dumps(d).encode()


if not getattr(_b2j, "_wsplit_patched", False):
    _orig_cbk = _b2j.compile_bir_kernel

    def _cbk(bir, *a, **k):
        return _orig_cbk(_split_waits_json(bir), *a, **k)

    _b2j.compile_bir_kernel = _cbk
    _b2j._wsplit_patched = True


# ---------------------------------------------------------------------------
# CPU-side structural prep (graph topology only, no float math)

def _prep(edge_index):
    src = np.asarray(edge_index[0], dtype=np.int64)
    dst = np.asarray(edge_index[1], dtype=np.int64)
    loop = np.arange(N, dtype=np.int64)
    src = np.concatenate([loop, src])
    dst = np.concatenate([loop, dst])

    deg = np.bincount(dst, minlength=N)

    perms = []        # per core: processing order (local node ids 0..NPC-1)
    blk_deg = np.zeros((NCORES, NBLK), dtype=np.int64)
    for k in range(NCORES):
        dk = deg[k * NPC:(k + 1) * NPC]
        order = np.argsort(-dk, kind="stable").astype(np.int64)
        perms.append(order)
        dks = np.concatenate([dk[order], np.zeros(ROWS - NPC, np.int64)])
        blk_deg[k] = dks.reshape(NBLK, P).max(1)

    # Uniform group structure across cores: W per block = max over cores,
    # then greedily merge consecutive blocks (pad to group max) keeping the
    # added padding under ~8% and the per-group volume bounded.
    wblk = blk_deg.max(0)          # [NBLK], non-increasing
    groups = []                    # list of (start_blk, end_blk, W)
    g0 = 0
    waste = 0.0
    real = 1.0
    for b in range(1, NBLK + 1):
        merge = False
        if b < NBLK:
            new_waste = waste + (wblk[g0] - wblk[b])
            new_real = real + wblk[b]
            vol = (b + 1 - g0) * max(wblk[g0], 1)
            if new_waste <= 0.08 * new_real and vol <= 256:
                merge = True
        if merge:
            waste, real = new_waste, new_real
            continue
        groups.append((g0, b, int(max(wblk[g0], 1))))
        if b < NBLK:
            g0 = b
            waste = 0.0
            real = float(wblk[b])
    col_off = np.zeros(NBLK, dtype=np.int64)
    C = 0
    for (a, b, w) in groups:
        for blk in range(a, b):
            col_off[blk] = C
            C += w
    if C % 4:
        C += 4 - C % 4          # keep idx16 wrapping aligned

    # permuted-global position of each node (for the h2 gather)
    pos2 = np.zeros(N, dtype=np.int64)
    for k in range(NCORES):
        inv = np.zeros(NPC, dtype=np.int64)
        inv[perms[k]] = np.arange(NPC)
        pos2[k * NPC:(k + 1) * NPC] = k * ROWS + inv

    pos_a = np.zeros((NCORES, P, C), dtype=np.int64)   # layer-1 gather pos
    pos_b = np.zeros((NCORES, P, C), dtype=np.int64)   # layer-2 gather pos
    valid = np.zeros((NCORES, P, C), dtype=bool)

    order = np.argsort(dst, kind="stable")
    src_s = src[order]
    dst_s = dst[order]
    starts = np.searchsorted(dst_s, np.arange(N + 1))

    for k in range(NCORES):
        inv_k = np.zeros(NPC, dtype=np.int64)
        inv_k[perms[k]] = np.arange(NPC)
        base = k * NPC
        lo, hi = starts[base], starts[base + NPC]
        d_loc = dst_s[lo:hi] - base                  # local dst id
        i_proc = inv_k[d_loc]                        # processing index
        blk = i_proc // P
        p = i_proc - blk * P
        seg_start = starts[d_loc + base] - lo        # rank within segment
        rank = np.arange(hi - lo) - seg_start
        c = col_off[blk] + rank
        pos_a[k, p, c] = src_s[lo:hi]
        pos_b[k, p, c] = pos2[src_s[lo:hi]]
        valid[k, p, c] = True

    maskneg = np.where(valid, 0.0, BIG_NEG).astype(np.float32)

    # ---- split columns: leading groups stay on the SWDGE path, the rest
    # go through the PE one-hot gather.  Self-loop (rank-0) columns are
    # written by the own-copy in both paths.
    skip = set()
    for (a, b, w) in groups:
        c0 = int(col_off[a])
        skip.update(c0 + i * w for i in range(b - a))
    all_cols = []
    for (a, b, w) in groups:
        c0 = int(col_off[a])
        for c in range(c0, c0 + w * (b - a)):
            if c not in skip:
                all_cols.append(c)
    pool_cols = all_cols[:POOL_COLS]
    pe_cols = all_cols[POOL_COLS:]
    n_pe = len(pe_cols)
    nb_pe = (n_pe + 31) // 32

    def pe_consts(pos):
        q = (pos[:, :, pe_cols] // TABL).astype(np.float16)   # [NC, P, n_pe]
        lane = (pos[:, :, pe_cols] % TABL).astype(np.float32)
        qrows = np.zeros((NCORES, 32, nb_pe * P), np.float16)
        for jb in range(nb_pe):
            blk = q[:, :, jb * 32:(jb + 1) * 32]              # [NC, P, <=32]
            qrows[:, :blk.shape[2], jb * P:jb * P + P] = blk.transpose(0, 2, 1)
        lanes = np.zeros((NCORES, P, max(n_pe, 1)), np.float32)
        lanes[:, :, :n_pe] = lane
        return qrows, lanes

    q1rows, lane1 = pe_consts(pos_a)
    q2rows, lane2 = pe_consts(pos_b)

    return {
        "groups": groups, "C": C, "perms": perms,
        "srcpa": pos_a.astype(np.int32), "srcpb": pos_b.astype(np.int32),
        "maskneg": maskneg,
        "pool_cols": pool_cols,
        "pe_cols": pe_cols, "nb_pe": nb_pe,
        "q1rows": q1rows, "lane1": lane1,
        "q2rows": q2rows, "lane2": lane2,
    }


# ---------------------------------------------------------------------------
# Bass program (identical for all cores; per-core data differs)

def _build(prep):
    C, groups = prep["C"], prep["groups"]
    n_pe = max(len(prep["pe_cols"]), 1)
    nb_pe = max(prep["nb_pe"], 1)
    nc = bass.Bass("TRN2", target_bir_lowering=False, debug=False,
                   num_devices=NCORES)
    xt = nc.dram_tensor("xt", [N, 1], F32, kind="ExternalInput").ap()
    x_own = nc.dram_tensor("x_own", [P, NBLK], F32, kind="ExternalInput").ap()
    idx16a = nc.dram_tensor("srcpa", [P, C], I32, kind="ExternalInput").ap()
    idx16b = nc.dram_tensor("srcpb", [P, C], I32, kind="ExternalInput").ap()
    mneg = nc.dram_tensor("mneg", [P, C], F32, kind="ExternalInput").ap()
    w1 = nc.dram_tensor("w1", [1, 128], F32, kind="ExternalInput").ap()
    as1 = nc.dram_tensor("as1", [1, 128], F32, kind="ExternalInput").ap()
    ad1 = nc.dram_tensor("ad1", [1, 128], F32, kind="ExternalInput").ap()
    b1 = nc.dram_tensor("b1", [1, 128], F32, kind="ExternalInput").ap()
    w2 = nc.dram_tensor("w2", [1, 128], F32, kind="ExternalInput").ap()
    sc2 = nc.dram_tensor("sc2", [1, 8], F32, kind="ExternalInput").ap()
    # sc2 row: [att_src2, att_dst2, b2, 0 | iota4]
    xtab = nc.dram_tensor("xtab", [256, TABL], F32R,
                          kind="ExternalInput").ap()
    q1r_d = nc.dram_tensor("q1rows", [32, nb_pe * P], F16,
                           kind="ExternalInput").ap()
    lane1_d = nc.dram_tensor("lane1", [P, n_pe], F32,
                             kind="ExternalInput").ap()
    q2r_d = nc.dram_tensor("q2rows", [32, nb_pe * P], F16,
                           kind="ExternalInput").ap()
    lane2_d = nc.dram_tensor("lane2", [P, n_pe], F32,
                             kind="ExternalInput").ap()
    oh32_d = nc.dram_tensor("oh32", [32, 32 * P], F16,
                            kind="ExternalInput").ap()
    iotal_d = nc.dram_tensor("iotal", [P, TABL], F16,
                             kind="ExternalInput").ap()
    iotap_d = nc.dram_tensor("iotap", [P, 2], F32, kind="ExternalInput").ap()

    out_d = nc.dram_tensor("out", [P, NBLK], F32, kind="ExternalOutput").ap()
    h2t_slice = nc.dram_tensor("h2t_slice", [ROWS, 1], F32,
                               kind="Internal").ap()
    h2t_full = nc.dram_tensor("h2t_full", [NCORES * ROWS, 1], F32,
                              kind="Internal", addr_space="Shared").ap()

    cfg = {
        "pool_cols": prep["pool_cols"], "pe_cols": prep["pe_cols"],
        "nb_pe": nb_pe,
        "xtab": xtab, "q1r_d": q1r_d, "lane1_d": lane1_d,
        "q2r_d": q2r_d, "lane2_d": lane2_d,
        "iotal_d": iotal_d, "iotap_d": iotap_d, "oh32_d": oh32_d,
    }
    with tile.TileContext(nc, num_cores=NCORES) as tc:
        _body(nc, tc, C, groups, xt, x_own, idx16a, idx16b,
              mneg, w1, as1, ad1, b1, w2, sc2, out_d, h2t_slice, h2t_full,
              cfg)
    return nc


def _gather_select(nc, tc, C, table_d, srcp, xs, tag, groups=None,
                   group_c0=None, own=None, pool_cols=()):
    """xs[p, c] = table[srcp[p, c]] via per-column indirect row gathers.

    Every node's rank-0 edge is its self-loop, so the first column of each
    block holds the node's own value: filled with a strided DVE copy instead
    of 128 DMA descriptors.  Only pool_groups' columns are gathered here;
    the rest are produced by _pe_gather."""
    skip = set()
    if groups is not None:
        for (a, b, w) in groups:
            nb = b - a
            c0 = group_c0[a]
            skip.update(c0 + i * w for i in range(nb))
            nc.vector.tensor_copy(
                out=xs[:, c0:c0 + nb * w:w], in_=own[:, a:b])
    for c in pool_cols:
        nc.gpsimd.indirect_dma_start(
            out=xs[:, c:c + 1], out_offset=None, in_=table_d,
            in_offset=bass.IndirectOffsetOnAxis(ap=srcp[:, c:c + 1],
                                                axis=0))


def _pe_gather(nc, tc, xs, qrows, lanes, tabA, tabB, iotal, iotap, oh32,
               pe_cols):
    """xs[p, pe_cols[j]] = tab[q[p, j], lane[p, j]] via one-hot f32r matmuls.

    Per column: broadcast the 128 q-values across partitions (ones-matmul),
    build the two [128, 128] one-hot chunks with is_equal vs a partition
    iota, row-gather the [256, TABL] table into PSUM with two chained f32r
    matmuls, then select the lane with a fused is_equal*mult and reduce."""
    import contextlib
    B = 6
    ctx = contextlib.ExitStack()
    with ctx:
        psb = ctx.enter_context(tc.tile_pool(name="psb", bufs=2, space="PSUM"))
        psy = ctx.enter_context(tc.tile_pool(name="psy", bufs=3, space="PSUM"))
        mk = ctx.enter_context(tc.tile_pool(name="mk", bufs=4))
        nb = len(pe_cols)
        for j0 in range(0, nb, B):
            cols = pe_cols[j0:j0 + B]
            m = len(cols)
            pb = psb.tile([P, B * P], F32, tag="pb")
            for i in range(m):
                j = j0 + i
                jb, jr = j // 32, j % 32
                nc.tensor.matmul(pb[:, i * P:(i + 1) * P],
                                 lhsT=oh32[:, jr * P:(jr + 1) * P],
                                 rhs=qrows[:, jb * P:(jb + 1) * P],
                                 start=True, stop=True)
            mask = mk.tile([P, 2 * B * P], F32R, tag="mk")
            nc.vector.tensor_scalar(out=mask[:, 0:m * P], in0=pb[:, 0:m * P],
                                    scalar1=iotap[:, 0:1], scalar2=None,
                                    op0=AT.is_equal)
            nc.vector.tensor_scalar(out=mask[:, B * P:B * P + m * P],
                                    in0=pb[:, 0:m * P],
                                    scalar1=iotap[:, 1:2], scalar2=None,
                                    op0=AT.is_equal)
            for i in range(m):
                j = j0 + i
                c = cols[i]
                py = psy.tile([P, TABL], F32, tag="py")
                nc.tensor.matmul(py[:], lhsT=mask[:, i * P:(i + 1) * P],
                                 rhs=tabA[:], start=True, stop=False)
                nc.tensor.matmul(py[:],
                                 lhsT=mask[:, B * P + i * P:B * P + (i + 1) * P],
                                 rhs=tabB[:], start=False, stop=True)
                tmp = mk.tile([P, TABL], F16, tag="tmp")
                nc.vector.scalar_tensor_tensor(
                    out=tmp[:], in0=iotal[:], scalar=lanes[:, j:j + 1],
                    in1=py[:], op0=AT.is_equal, op1=AT.mult)
                dum = mk.tile([P, TABL], F16, tag="dum")
                nc.scalar.activation(out=dum[:], in_=tmp[:], func=AF.Copy,
                                     accum_out=xs[:, c:c + 1])


def _body(nc, tc, C, groups, xt_d, x_own_d, srcpa_d, srcpb_d,
          mneg_d, w1_d, as1_d, ad1_d, b1_d, w2_d, sc2_d,
          out_d, h2t_slice, h2t_full, cfg):
    import contextlib
    ctx = contextlib.ExitStack()
    H = 8
    with ctx:
        const = ctx.enter_context(tc.tile_pool(name="const", bufs=1))
        group_c0 = {}
        _c = 0
        for (ga, gb, gw) in groups:
            group_c0[ga] = _c
            _c += gw * (gb - ga)

        # ---- persistent loads
        mneg = const.tile([P, C], F32)
        nc.sync.dma_start(mneg[:], mneg_d[:])
        srcpa = const.tile([P, C], mybir.dt.int32)
        nc.sync.dma_start(srcpa[:], srcpa_d[:])
        x_own = const.tile([P, NBLK], F32)
        nc.sync.dma_start(x_own[:], x_own_d[:])

        # ---- params: one row, then broadcast via ones-matmul
        # 0:128 w1 | 128:256 as1 | 256:384 ad1 | 384:512 b1 | 512:640 w2
        # 640:648 sc2 (att_src2, att_dst2, b2, w2sum | iota4)
        # 648:656 s1 | 656:664 d1
        prow = const.tile([1, 664], F32)
        nc.sync.dma_start(prow[:, 0:128], w1_d[:])
        nc.sync.dma_start(prow[:, 128:256], as1_d[:])
        nc.sync.dma_start(prow[:, 256:384], ad1_d[:])
        nc.sync.dma_start(prow[:, 384:512], b1_d[:])
        nc.sync.dma_start(prow[:, 512:640], w2_d[:])
        nc.sync.dma_start(prow[:, 640:648], sc2_d[:])
        tmp = const.tile([1, 256], F32)
        nc.vector.tensor_tensor(out=tmp[:, 0:128], in0=prow[:, 0:128],
                                in1=prow[:, 128:256], op=AT.mult)
        nc.vector.tensor_tensor(out=tmp[:, 128:256], in0=prow[:, 0:128],
                                in1=prow[:, 256:384], op=AT.mult)
        nc.vector.tensor_reduce(out=prow[:, 648:664],
                                in_=tmp[:].rearrange("a (h c) -> a h c", c=16),
                                axis=mybir.AxisListType.X, op=AT.add)
        nc.vector.tensor_reduce(out=prow[:, 643:644], in_=prow[:, 512:640],
                                axis=mybir.AxisListType.X, op=AT.add)

        ones = const.tile([1, P], F32)
        nc.vector.memset(ones[:], 1.0)
        # funnel prow through one DVE copy so the matmul (whose load-weights
        # encoding has a tight sem-wait budget) depends on a single producer
        prow2 = const.tile([1, 664], F32)
        nc.vector.tensor_copy(out=prow2[:], in_=prow[:])
        pc = const.tile([P, 664], F32)
        with tc.tile_pool(name="psum", bufs=2, space="PSUM") as psum:
            for lo, hi in ((0, 512), (512, 664)):
                pcast = psum.tile([P, 512], F32, tag="pcast")
                nc.tensor.matmul(pcast[:, :hi - lo], lhsT=ones[:],
                                 rhs=prow2[:, lo:hi], start=True, stop=True)
                nc.vector.tensor_copy(out=pc[:, lo:hi],
                                      in_=pcast[:, :hi - lo])
        W1t = pc[:, 0:128]
        B1t = pc[:, 384:512]
        W2t = pc[:, 512:640]
        s2c = pc[:, 640:641]
        d2c = pc[:, 641:642]
        b2c = pc[:, 642:643]
        w2sum = pc[:, 643:644]
        iota4 = pc[:, 644:648]
        s1c = pc[:, 648:656]
        d1c = pc[:, 656:664]

        # ---- PE-gather constants
        n_pe = max(len(cfg["pe_cols"]), 1)
        qr1 = const.tile([32, cfg["nb_pe"] * P], F16)
        nc.sync.dma_start(qr1[:], cfg["q1r_d"][:])
        ln1 = const.tile([P, n_pe], F32)
        nc.sync.dma_start(ln1[:], cfg["lane1_d"][:])
        qr2 = const.tile([32, cfg["nb_pe"] * P], F16)
        nc.sync.dma_start(qr2[:], cfg["q2r_d"][:])
        ln2 = const.tile([P, n_pe], F32)
        nc.sync.dma_start(ln2[:], cfg["lane2_d"][:])
        oh32 = const.tile([32, 32 * P], F16)
        nc.sync.dma_start(oh32[:], cfg["oh32_d"][:])
        iotal = const.tile([P, TABL], F16)
        nc.sync.dma_start(iotal[:], cfg["iotal_d"][:])
        iotap = const.tile([P, 2], F32)
        nc.sync.dma_start(iotap[:], cfg["iotap_d"][:])
        tabA = const.tile([P, TABL], F32R)
        nc.sync.dma_start(tabA[:], cfg["xtab"][0:P, :])
        tabB = const.tile([P, TABL], F32R)
        nc.sync.dma_start(tabB[:], cfg["xtab"][P:256, :])

        # ---- gather x[src] (layer 1)
        xs = const.tile([P, C], F32)
        _gather_select(nc, tc, C, xt_d[:], srcpa, xs, "a",
                       groups=groups, group_c0=group_c0, own=x_own,
                       pool_cols=cfg["pool_cols"])
        if cfg["pe_cols"]:
            _pe_gather(nc, tc, xs, qr1, ln1, tabA, tabB, iotal, iotap,
                       oh32, cfg["pe_cols"])

        # adst[p, b, h] = x_own[p, b] * d1[h]
        adst = const.tile([P, NBLK * H], F32)
        nc.vector.tensor_tensor(
            out=adst[:].rearrange("p (b h) -> p b h", h=H),
            in0=x_own[:].rearrange("p b -> p b ()").to_broadcast([P, NBLK, H]),
            in1=d1c.rearrange("p h -> p () h").to_broadcast([P, NBLK, H]),
            op=AT.mult)

        denom = const.tile([P, NBLK * H], F32)
        zt = const.tile([P, NBLK * H], F32)

        # ---- layer-1 main, one run per block-group
        with tc.tile_pool(name="work", bufs=2) as work:
            _layer1_main(nc, C, groups, group_c0, work, xs, mneg, adst, s1c,
                         denom, zt)

        # ---- layer-1 epilogue -> h2_own [P, NBLK]
        r = const.tile([P, NBLK * H], F32)
        nc.vector.tensor_scalar(out=r[:], in0=denom[:], scalar1=float(EPS),
                                scalar2=None, op0=AT.add)
        nc.vector.reciprocal(out=r[:], in_=r[:])
        nc.vector.tensor_tensor(out=r[:], in0=r[:], in1=zt[:], op=AT.mult)

        h2_own = const.tile([P, NBLK], F32)
        with tc.tile_pool(name="ep", bufs=2) as ep:
            _epilogue(nc, ep, r, W1t, B1t, W2t, h2_own)
        nc.vector.tensor_scalar(out=h2_own[:], in0=h2_own[:], scalar1=w2sum,
                                scalar2=None, op0=AT.subtract)
        pe2 = {"qr": qr2, "ln": ln2, "iotal": iotal, "iotap": iotap,
               "oh32": oh32, "pe_cols": cfg["pe_cols"],
               "pool_cols": cfg["pool_cols"]}
        _rest(nc, tc, C, groups, group_c0, const, mneg, h2_own, srcpa,
              srcpb_d, s2c, d2c, b2c, out_d, h2t_slice, h2t_full, pe2)


def _layer1_main(nc, C, groups, group_c0, work, xs, mneg, adst, s1c,
                 denom, zt):
        H = 8
        for (a, b, w) in groups:
            nb = b - a
            c0 = group_c0[a]
            V = nb * H * w
            xs_g = xs[:, c0:c0 + nb * w].rearrange("p (n w) -> p n () w", w=w)
            mn_g = mneg[:, c0:c0 + nb * w].rearrange("p (n w) -> p n () w", w=w)
            ad_g = adst[:, a * H:b * H].rearrange("p (n h) -> p n h ()", h=H)
            s1_g = s1c.rearrange("p h -> p () h ()")

            u = work.tile([P, V], F32, tag="u")
            u4 = u[:].rearrange("p (n h w) -> p n h w", h=H, w=w)
            nc.vector.tensor_tensor(out=u4, in0=xs_g.to_broadcast([P, nb, H, w]),
                                    in1=s1_g.to_broadcast([P, nb, H, w]), op=AT.mult)
            u2 = work.tile([P, V], F32, tag="u2")
            u24 = u2[:].rearrange("p (n h w) -> p n h w", h=H, w=w)
            nc.vector.tensor_tensor(out=u24, in0=u4,
                                    in1=ad_g.to_broadcast([P, nb, H, w]), op=AT.add)
            nc.vector.tensor_tensor(out=u4, in0=u24,
                                    in1=mn_g.to_broadcast([P, nb, H, w]), op=AT.add)
            # leaky relu: max(0.2*v, v), then exp
            if USE_ACT_LRELU:
                nc.scalar.activation(out=u24, in_=u4, func=AF.Lrelu,
                                     alpha=NEG_SLOPE)
            else:
                nc.vector.scalar_tensor_tensor(out=u24, in0=u4, scalar=NEG_SLOPE,
                                               in1=u4, op0=AT.mult, op1=AT.max)
            ex = work.tile([P, V], F32, tag="ex")
            ex4 = ex[:].rearrange("p (n h w) -> p n h w", h=H, w=w)
            nc.scalar.activation(out=ex4, in_=u24, func=AF.Exp)
            nc.vector.tensor_reduce(
                out=denom[:, a * H:b * H].rearrange("p (n h) -> p n h", h=H),
                in_=ex4, axis=mybir.AxisListType.X, op=AT.add)
            nc.vector.tensor_tensor(out=u4, in0=ex4,
                                    in1=xs_g.to_broadcast([P, nb, H, w]), op=AT.mult)
            nc.vector.tensor_reduce(
                out=zt[:, a * H:b * H].rearrange("p (n h) -> p n h", h=H),
                in_=u4, axis=mybir.AxisListType.X, op=AT.add)

def _epilogue(nc, ep, r, W1t, B1t, W2t, h2_own):
        H = 8
        EPB = 14
        for a in range(0, NBLK, EPB):
            b = min(a + EPB, NBLK)
            nb = b - a
            V = nb * 128
            v = ep.tile([P, EPB * 128], F32, tag="v")
            v4 = v[:, :V].rearrange("p (n h c) -> p n h c", h=H, c=16)
            r_g = r[:, a * H:b * H].rearrange("p (n h) -> p n h ()", h=H)
            w1_g = W1t.rearrange("p (h c) -> p () h c", c=16)
            b1_g = B1t.rearrange("p (h c) -> p () h c", c=16)
            nc.vector.tensor_tensor(out=v4, in0=r_g.to_broadcast([P, nb, H, 16]),
                                    in1=w1_g.to_broadcast([P, nb, H, 16]),
                                    op=AT.mult)
            v2 = ep.tile([P, EPB * 128], F32, tag="v2")
            nc.vector.tensor_tensor(
                out=v2[:, :V].rearrange("p (n h c) -> p n h c", h=H, c=16),
                in0=v4, in1=b1_g.to_broadcast([P, nb, H, 16]), op=AT.add)
            # h1' = max(v,0) + min(exp(v),1);  elu(v) = h1' - 1
            ev = ep.tile([P, EPB * 128], F32, tag="ev")
            nc.scalar.activation(out=ev[:, :V], in_=v2[:, :V], func=AF.Exp)
            nc.vector.tensor_scalar(out=ev[:, :V], in0=ev[:, :V], scalar1=1.0,
                                    scalar2=None, op0=AT.min)
            nc.vector.tensor_scalar(out=v2[:, :V], in0=v2[:, :V], scalar1=0.0,
                                    scalar2=None, op0=AT.max)
            nc.vector.tensor_tensor(out=v2[:, :V], in0=v2[:, :V], in1=ev[:, :V],
                                    op=AT.add)
            # h2 = sum h1'*W2 - W2sum  (the elu -1 folded into W2sum)
            w2_g = W2t.rearrange("p (h c) -> p () (h c)", c=16)
            nc.vector.tensor_tensor(
                out=v4, in0=v2[:, :V].rearrange("p (n f) -> p n f", f=128),
                in1=w2_g.to_broadcast([P, nb, 128]), op=AT.mult)
            nc.vector.tensor_reduce(
                out=h2_own[:, a:b], in_=v4.rearrange("p n h c -> p n (h c)"),
                axis=mybir.AxisListType.X, op=AT.add)
def _rest(nc, tc, C, groups, group_c0, const, mneg, h2_own, srcpa,
          srcpb_d, s2c, d2c, b2c, out_d, h2t_slice, h2t_full, pe2):
        # ---- store own h2 slice, AllGather the table
        nc.sync.dma_start(
            h2t_slice[:].rearrange("q l -> (q l)").rearrange(
                "(b p) -> p b", p=P),
            h2_own[:])
        nc.gpsimd.collective_compute(
            "AllGather", AT.bypass,
            replica_groups=[list(range(NCORES))],
            ins=[h2t_slice[:]], outs=[h2t_full[:]])

        # ---- layer-2 PE tables from the gathered h2 (rounded to f32r)
        h2tab_f = const.tile([P, 2 * TABL], F32)
        h2t2d = h2t_full[:].rearrange("(q l) one -> q (l one)", q=256)
        nc.sync.dma_start(h2tab_f[:, 0:TABL], h2t2d[0:P, :])
        nc.sync.dma_start(h2tab_f[:, TABL:2 * TABL], h2t2d[P:256, :])
        h2tabA = const.tile([P, TABL], F32R)
        nc.vector.tensor_copy(out=h2tabA[:], in_=h2tab_f[:, 0:TABL])
        h2tabB = const.tile([P, TABL], F32R)
        nc.vector.tensor_copy(out=h2tabB[:], in_=h2tab_f[:, TABL:2 * TABL])

        # ---- layer 2: reuse the srcpa tile's SBUF for srcpb
        nc.sync.dma_start(srcpa[:], srcpb_d[:])
        h2s = const.tile([P, C], F32)
        _gather_select(nc, tc, C, h2t_full[:], srcpa, h2s, "b",
                       groups=groups, group_c0=group_c0, own=h2_own,
                       pool_cols=pe2["pool_cols"])
        if pe2["pe_cols"]:
            _pe_gather(nc, tc, h2s, pe2["qr"], pe2["ln"], h2tabA, h2tabB,
                       pe2["iotal"], pe2["iotap"], pe2["oh32"],
                       pe2["pe_cols"])

        adst2 = const.tile([P, NBLK], F32)
        nc.vector.tensor_scalar(out=adst2[:], in0=h2_own[:], scalar1=d2c,
                                scalar2=None, op0=AT.mult)

        den2 = const.tile([P, NBLK], F32)
        z2 = const.tile([P, NBLK], F32)
        with tc.tile_pool(name="work2", bufs=2) as work:
            _layer2_main(nc, groups, group_c0, work, h2s, mneg, adst2, s2c,
                         den2, z2)
        _output(nc, den2, z2, b2c, out_d)


def _layer2_main(nc, groups, group_c0, work, h2s, mneg, adst2, s2c, den2, z2):
        for (a, b, w) in groups:
            nb = b - a
            c0 = group_c0[a]
            V = nb * w
            sl = slice(c0, c0 + V)
            h2s_g = h2s[:, sl].rearrange("p (n w) -> p n w", w=w)
            u = work.tile([P, V], F32, tag="u")
            u3 = u[:].rearrange("p (n w) -> p n w", w=w)
            nc.vector.scalar_tensor_tensor(
                out=u3, in0=h2s_g, scalar=s2c,
                in1=adst2[:, a:b].rearrange("p n -> p n ()").to_broadcast(
                    [P, nb, w]),
                op0=AT.mult, op1=AT.add)
            u2 = work.tile([P, V], F32, tag="u2")
            u23 = u2[:].rearrange("p (n w) -> p n w", w=w)
            nc.vector.tensor_tensor(
                out=u23, in0=u3,
                in1=mneg[:, sl].rearrange("p (n w) -> p n w", w=w), op=AT.add)
            if USE_ACT_LRELU:
                nc.scalar.activation(out=u3, in_=u23, func=AF.Lrelu,
                                     alpha=NEG_SLOPE)
            else:
                nc.vector.scalar_tensor_tensor(out=u3, in0=u23, scalar=NEG_SLOPE,
                                               in1=u23, op0=AT.mult, op1=AT.max)
            nc.scalar.activation(out=u23, in_=u3, func=AF.Exp)
            nc.vector.tensor_reduce(out=den2[:, a:b], in_=u23,
                                    axis=mybir.AxisListType.X, op=AT.add)
            nc.vector.tensor_tensor(out=u3, in0=u23, in1=h2s_g, op=AT.mult)
            nc.vector.tensor_reduce(out=z2[:, a:b], in_=u3,
                                    axis=mybir.AxisListType.X, op=AT.add)


def _output(nc, den2, z2, b2c, out_d):
        # ---- output
        nc.vector.tensor_scalar(out=den2[:], in0=den2[:], scalar1=float(EPS),
                                scalar2=None, op0=AT.add)
        nc.vector.reciprocal(out=den2[:], in_=den2[:])
        nc.vector.tensor_tensor(out=z2[:], in0=z2[:], in1=den2[:], op=AT.mult)
        nc.vector.tensor_scalar(out=z2[:], in0=z2[:], scalar1=b2c,
                                scalar2=None, op0=AT.add)
        nc.sync.dma_start(out_d[:], z2[:])


# ---------------------------------------------------------------------------

def kernel(**inputs):
    edge_index = np.asarray(inputs["edge_index"])
    prep = _prep(edge_index)
    C, groups, perms = prep["C"], prep["groups"], prep["perms"]

    x = np.asarray(inputs["x"], dtype=np.float32).reshape(-1)   # [N]
    xt = np.ascontiguousarray(x.reshape(N, 1))

    flat = lambda a: np.ascontiguousarray(
        np.asarray(a, dtype=np.float32).reshape(1, -1))
    w1 = flat(inputs["W1"]); as1 = flat(inputs["att_src1"])
    ad1 = flat(inputs["att_dst1"]); b1 = flat(inputs["b1"])
    w2 = flat(inputs["W2"])
    sc2 = np.zeros((1, 8), np.float32)
    sc2[0, 0] = np.asarray(inputs["att_src2"]).reshape(-1)[0]
    sc2[0, 1] = np.asarray(inputs["att_dst2"]).reshape(-1)[0]
    sc2[0, 2] = np.asarray(inputs["b2"]).reshape(-1)[0]
    sc2[0, 4:8] = [0.0, 1.0, 2.0, 3.0]

    nc = _build(prep)

    xtab = np.zeros(256 * TABL, np.float32)
    xtab[:N] = x
    xtab = np.ascontiguousarray(xtab.reshape(256, TABL))
    iotal = np.broadcast_to(np.arange(TABL, dtype=np.float16), (P, TABL))
    iotal = np.ascontiguousarray(iotal)
    iotap = np.stack([np.arange(P, dtype=np.float32),
                      np.arange(P, dtype=np.float32) + P], axis=1)
    iotap = np.ascontiguousarray(iotap)
    oh32 = np.zeros((32, 32, P), np.float16)
    for j in range(32):
        oh32[j, j, :] = 1.0
    oh32 = np.ascontiguousarray(oh32.reshape(32, 32 * P))

    in_maps = []
    for k in range(NCORES):
        xk = x[k * NPC:(k + 1) * NPC][perms[k]]
        xk = np.concatenate([xk, np.zeros(ROWS - NPC, np.float32)])
        x_own = np.ascontiguousarray(xk.reshape(NBLK, P).T)
        in_maps.append({
            "xt": xt, "x_own": x_own,
            "srcpa": prep["srcpa"][k], "srcpb": prep["srcpb"][k],
            "mneg": prep["maskneg"][k],
            "w1": w1, "as1": as1, "ad1": ad1, "b1": b1, "w2": w2, "sc2": sc2,
            "xtab": xtab,
            "q1rows": np.ascontiguousarray(prep["q1rows"][k]),
            "lane1": np.ascontiguousarray(prep["lane1"][k]),
            "q2rows": np.ascontiguousarray(prep["q2rows"][k]),
            "lane2": np.ascontiguousarray(prep["lane2"][k]),
            "iotal": iotal, "iotap": iotap, "oh32": oh32,
        })

    res = run_bass_kernel_spmd(nc, in_maps, core_ids=list(range(NCORES)))

    out = np.zeros((N, 1), np.float32)
    for k in range(NCORES):
        o = res.results[k]["out"]                    # [P, NBLK]
        flat_o = o.T.reshape(-1)[:NPC]
        out[k * NPC:(k + 1) * NPC, 0][perms[k]] = flat_o
    return out



# revision 28
# speedup vs baseline: 1.1111x; 1.1111x over previous
"""GAT (2-layer, PyG-style) on 8 Trainium2 NeuronCores.

Approach
--------
Layer 1 has in_channels=1, so h = x @ W1 is rank-1: every per-edge quantity
reduces to scalars per node.  With s1[h] = sum_c W1[h,c]*att_src1[h,c] and
d1[h] = sum_c W1[h,c]*att_dst1[h,c]:

    e[i,h]   = leaky_relu(s1[h]*x[src_i] + d1[h]*x[dst_i])
    denom[d,h] = sum_{i->d} exp(e[i,h])          (max-shift skipped: |e| < ~10)
    z[d,h]     = sum_{i->d} exp(e[i,h]) * x[src_i]
    out1[d,h,c] = elu(W1[h,c]*z/(denom+eps) + b1[h,c])

Layer 2 (heads=1, out=1) similarly only needs the scalar h2 = out1 @ W2.

Sharding: dst-owner node sharding (12500 nodes/core).  Per core, nodes are
degree-sorted and packed into 98 blocks of 128 (partition dim); each node's
incoming edges occupy W columns (block-group-padded).  x[dst] is then a free
per-partition broadcast and segment sums are row reductions; only x[src]
needs a real gather.

The per-edge scalar gather is split across two engine paths that run
concurrently (POOL_COLS tunes the balance):
  - SWDGE path: per-column indirect DMAs (128 per-partition descriptors per
    instruction; one instruction can only carry one offset per partition on
    this walrus, and ~1us/instruction is the floor).
  - PE path: the table lives in SBUF as [256, 392] float32r; per slot
    column, the 128 q-values are broadcast across partitions with a one-hot
    row-select matmul, turned into two [128, 128] one-hot chunks with
    is_equal vs a partition iota (batched 4 columns per DVE op), row-gathered
    with two chained f32r matmuls into PSUM, and the lane is selected with a
    fused is_equal*mult (DVE) + accumulate-reduce on the Activation engine
    (accum_out).  f32r rounds the table to ~11 mantissa bits and the select
    products go through f16, final rel err ~9e-3 (tolerance 2e-2).

h2 is exchanged between layers with an AllGather of the per-core slices;
both layers use the same [256, 392] table shape (8*12544 = 100352 = 256*392).
Baseline TimelineSim 9.09 ms (all-SWDGE, 1 DMA sem) -> 1.84 ms here
(NUM_SWDGE_GLOBAL_SEMS=8 + hybrid gather).  (The dedicated
DMA-gather/ap-gather custom instructions do not compile on this toolchain
-- "ISA wrong length" even for load_library -- and walrus only encodes ~2
sem waits per instruction, hence the BIR post-pass below that splits excess
waits onto injected NoOps.)
"""
import sys
sys.path.insert(0, "/opt/trn_rl_repo")
import re
import numpy as np
import concourse.bass as bass
import concourse.mybir as mybir
import concourse.tile as tile
from concourse import library_config
from concourse.bass_utils import run_bass_kernel_spmd
from bass_rust import ScopedClock, VectorClock

N = 100000
NCORES = 8
NPC = N // NCORES          # nodes per core
P = 128
NBLK = (NPC + P - 1) // P  # 98
ROWS = NBLK * P            # 12544
NEG_SLOPE = 0.2
EPS = 1e-16
BIG_NEG = -1.0e30
USE_ACT_LRELU = False
GCHUNK = 96                # gather chunk, in slot-columns
TABL = 392                 # PE-gather table lane count ([256, 392] = 100352)
POOL_COLS = 760            # columns kept on the SWDGE indirect-DMA path

F32 = mybir.dt.float32
F32R = mybir.dt.float32r
F16 = mybir.dt.float16
I16 = mybir.dt.int16
I32 = mybir.dt.int32
AT = mybir.AluOpType
AF = mybir.ActivationFunctionType


# ---------------------------------------------------------------------------
# Tile tail-drain workaround: walrus TPB_CTRL codegen rejects a Drain with
# more than two sem waits; emit one NOP-wait per proc first.
def _split_drain_and_barrier(self, tick_clock, wait_clock):
    gc = tick_clock.global_clock
    ticks = [int(x) for x in re.findall(r"\d+", repr(gc))]
    for i, t in enumerate(ticks):
        if t <= 0:
            continue
        sub = VectorClock()
        sub.require_at_least(i, t)
        inst = self.nc.sync.nop()
        wait_clock.add_sem_waits(inst.ins, ScopedClock({None: sub}))
    self.nc.sync.drain()
    self.nc.all_engine_barrier()
    popped = self.nc._tile_sem_poison_stack.pop()
    assert popped is self._sem_poison
    self.nc.clear_and_free_semaphores(list(self.sems.allocated().values()))
    self.nc.all_engine_barrier()


tile.TileContext._drain_and_barrier = _split_drain_and_barrier

import concourse.tile_sem_assignment as _tsa
_tsa.NUM_SWDGE_GLOBAL_SEMS = 8
_tsa.NUM_HWDGE_SEMS = 8

# Walrus encodes at most ~2 sem waits per instruction; split any excess onto
# injected same-engine NoOps in the BIR JSON right before compilation.
import json as _json
from concourse import bass2jax as _b2j


def _split_waits_json(bir, max_keep=1):
    d = _json.loads(bir)
    ctr = [0]

    def fix_block(blk):
        out = []
        for inst in blk.get("instructions", []):
            si = inst.get("sync_info")
            waits = (si or {}).get("on_wait") or []
            if len(waits) > max_keep and inst.get("opcode") != "NoOp":
                keep = waits[-max_keep:]
                for w in waits[:-max_keep]:
                    ctr[0] += 1
                    out.append({"debug": inst.get("debug", 0),
                                "engine": inst["engine"], "ins": [],
                                "outs": [], "name": f"I-wsp{ctr[0]}",
                                "opcode": "NoOp",
                                "sync_info": {"on_update": [], "on_wait": [w]}})
                si["on_wait"] = keep
            out.append(inst)
        blk["instructions"] = out
        for sb in blk.get("blocks", []):
            fix_block(sb)

    for fn in d["functions"]:
        for blk in fn["blocks"]:
            fix_block(blk)
    return _json.dumps(d).encode()


if not getattr(_b2j, "_wsplit_patched", False):
    _orig_cbk = _b2j.compile_bir_kernel

    def _cbk(bir, *a, **k):
        return _orig_cbk(_split_waits_json(bir), *a, **k)

    _b2j.compile_bir_kernel = _cbk
    _b2j._wsplit_patched = True


# ---------------------------------------------------------------------------
# CPU-side structural prep (graph topology only, no float math)

def _prep(edge_index):
    src = np.asarray(edge_index[0], dtype=np.int64)
    dst = np.asarray(edge_index[1], dtype=np.int64)
    loop = np.arange(N, dtype=np.int64)
    src = np.concatenate([loop, src])
    dst = np.concatenate([loop, dst])

    deg = np.bincount(dst, minlength=N)

    perms = []        # per core: processing order (local node ids 0..NPC-1)
    blk_deg = np.zeros((NCORES, NBLK), dtype=np.int64)
    for k in range(NCORES):
        dk = deg[k * NPC:(k + 1) * NPC]
        order = np.argsort(-dk, kind="stable").astype(np.int64)
        perms.append(order)
        dks = np.concatenate([dk[order], np.zeros(ROWS - NPC, np.int64)])
        blk_deg[k] = dks.reshape(NBLK, P).max(1)

    # Uniform group structure across cores: W per block = max over cores,
    # then greedily merge consecutive blocks (pad to group max) keeping the
    # added padding under ~8% and the per-group volume bounded.
    wblk = blk_deg.max(0)          # [NBLK], non-increasing
    groups = []                    # list of (start_blk, end_blk, W)
    g0 = 0
    waste = 0.0
    real = 1.0
    for b in range(1, NBLK + 1):
        merge = False
        if b < NBLK:
            new_waste = waste + (wblk[g0] - wblk[b])
            new_real = real + wblk[b]
            vol = (b + 1 - g0) * max(wblk[g0], 1)
            if new_waste <= 0.08 * new_real and vol <= 256:
                merge = True
        if merge:
            waste, real = new_waste, new_real
            continue
        groups.append((g0, b, int(max(wblk[g0], 1))))
        if b < NBLK:
            g0 = b
            waste = 0.0
            real = float(wblk[b])
    col_off = np.zeros(NBLK, dtype=np.int64)
    C = 0
    for (a, b, w) in groups:
        for blk in range(a, b):
            col_off[blk] = C
            C += w
    if C % 4:
        C += 4 - C % 4          # keep idx16 wrapping aligned

    # permuted-global position of each node (for the h2 gather)
    pos2 = np.zeros(N, dtype=np.int64)
    for k in range(NCORES):
        inv = np.zeros(NPC, dtype=np.int64)
        inv[perms[k]] = np.arange(NPC)
        pos2[k * NPC:(k + 1) * NPC] = k * ROWS + inv

    pos_a = np.zeros((NCORES, P, C), dtype=np.int64)   # layer-1 gather pos
    pos_b = np.zeros((NCORES, P, C), dtype=np.int64)   # layer-2 gather pos
    valid = np.zeros((NCORES, P, C), dtype=bool)

    order = np.argsort(dst, kind="stable")
    src_s = src[order]
    dst_s = dst[order]
    starts = np.searchsorted(dst_s, np.arange(N + 1))

    for k in range(NCORES):
        inv_k = np.zeros(NPC, dtype=np.int64)
        inv_k[perms[k]] = np.arange(NPC)
        base = k * NPC
        lo, hi = starts[base], starts[base + NPC]
        d_loc = dst_s[lo:hi] - base                  # local dst id
        i_proc = inv_k[d_loc]                        # processing index
        blk = i_proc // P
        p = i_proc - blk * P
        seg_start = starts[d_loc + base] - lo        # rank within segment
        rank = np.arange(hi - lo) - seg_start
        c = col_off[blk] + rank
        pos_a[k, p, c] = src_s[lo:hi]
        pos_b[k, p, c] = pos2[src_s[lo:hi]]
        valid[k, p, c] = True

    maskneg = np.where(valid, 0.0, BIG_NEG).astype(np.float32)

    # ---- split columns: leading groups stay on the SWDGE path, the rest
    # go through the PE one-hot gather.  Self-loop (rank-0) columns are
    # written by the own-copy in both paths.
    skip = set()
    for (a, b, w) in groups:
        c0 = int(col_off[a])
        skip.update(c0 + i * w for i in range(b - a))
    all_cols = []
    for (a, b, w) in groups:
        c0 = int(col_off[a])
        for c in range(c0, c0 + w * (b - a)):
            if c not in skip:
                all_cols.append(c)
    pool_cols = all_cols[:POOL_COLS]
    pe_cols = all_cols[POOL_COLS:]
    n_pe = len(pe_cols)
    nb_pe = (n_pe + 31) // 32

    def pe_consts(pos):
        q = (pos[:, :, pe_cols] // TABL).astype(np.float16)   # [NC, P, n_pe]
        lane = (pos[:, :, pe_cols] % TABL).astype(np.float32)
        qrows = np.zeros((NCORES, 32, nb_pe * P), np.float16)
        for jb in range(nb_pe):
            blk = q[:, :, jb * 32:(jb + 1) * 32]              # [NC, P, <=32]
            qrows[:, :blk.shape[2], jb * P:jb * P + P] = blk.transpose(0, 2, 1)
        lanes = np.zeros((NCORES, P, max(n_pe, 1)), np.float32)
        lanes[:, :, :n_pe] = lane
        return qrows, lanes

    q1rows, lane1 = pe_consts(pos_a)
    q2rows, lane2 = pe_consts(pos_b)

    return {
        "groups": groups, "C": C, "perms": perms,
        "srcpa": pos_a.astype(np.int32), "srcpb": pos_b.astype(np.int32),
        "maskneg": maskneg,
        "pool_cols": pool_cols,
        "pe_cols": pe_cols, "nb_pe": nb_pe,
        "q1rows": q1rows, "lane1": lane1,
        "q2rows": q2rows, "lane2": lane2,
    }


# ---------------------------------------------------------------------------
# Bass program (identical for all cores; per-core data differs)

def _build(prep):
    C, groups = prep["C"], prep["groups"]
    n_pe = max(len(prep["pe_cols"]), 1)
    nb_pe = max(prep["nb_pe"], 1)
    nc = bass.Bass("TRN2", target_bir_lowering=False, debug=False,
                   num_devices=NCORES)
    xt = nc.dram_tensor("xt", [N, 1], F32, kind="ExternalInput").ap()
    x_own = nc.dram_tensor("x_own", [P, NBLK], F32, kind="ExternalInput").ap()
    idx16a = nc.dram_tensor("srcpa", [P, C], I32, kind="ExternalInput").ap()
    idx16b = nc.dram_tensor("srcpb", [P, C], I32, kind="ExternalInput").ap()
    mneg = nc.dram_tensor("mneg", [P, C], F32, kind="ExternalInput").ap()
    w1 = nc.dram_tensor("w1", [1, 128], F32, kind="ExternalInput").ap()
    as1 = nc.dram_tensor("as1", [1, 128], F32, kind="ExternalInput").ap()
    ad1 = nc.dram_tensor("ad1", [1, 128], F32, kind="ExternalInput").ap()
    b1 = nc.dram_tensor("b1", [1, 128], F32, kind="ExternalInput").ap()
    w2 = nc.dram_tensor("w2", [1, 128], F32, kind="ExternalInput").ap()
    sc2 = nc.dram_tensor("sc2", [1, 8], F32, kind="ExternalInput").ap()
    # sc2 row: [att_src2, att_dst2, b2, 0 | iota4]
    xtab = nc.dram_tensor("xtab", [256, TABL], F32R,
                          kind="ExternalInput").ap()
    q1r_d = nc.dram_tensor("q1rows", [32, nb_pe * P], F16,
                           kind="ExternalInput").ap()
    lane1_d = nc.dram_tensor("lane1", [P, n_pe], F32,
                             kind="ExternalInput").ap()
    q2r_d = nc.dram_tensor("q2rows", [32, nb_pe * P], F16,
                           kind="ExternalInput").ap()
    lane2_d = nc.dram_tensor("lane2", [P, n_pe], F32,
                             kind="ExternalInput").ap()
    oh32_d = nc.dram_tensor("oh32", [32, 32 * P], F16,
                            kind="ExternalInput").ap()
    iotal_d = nc.dram_tensor("iotal", [P, TABL], F16,
                             kind="ExternalInput").ap()
    iotap_d = nc.dram_tensor("iotap", [P, 2], F32, kind="ExternalInput").ap()

    out_d = nc.dram_tensor("out", [P, NBLK], F32, kind="ExternalOutput").ap()
    h2t_slice = nc.dram_tensor("h2t_slice", [ROWS, 1], F32,
                               kind="Internal").ap()
    h2t_full = nc.dram_tensor("h2t_full", [NCORES * ROWS, 1], F32,
                              kind="Internal", addr_space="Shared").ap()

    cfg = {
        "pool_cols": prep["pool_cols"], "pe_cols": prep["pe_cols"],
        "nb_pe": nb_pe,
        "xtab": xtab, "q1r_d": q1r_d, "lane1_d": lane1_d,
        "q2r_d": q2r_d, "lane2_d": lane2_d,
        "iotal_d": iotal_d, "iotap_d": iotap_d, "oh32_d": oh32_d,
    }
    with tile.TileContext(nc, num_cores=NCORES) as tc:
        _body(nc, tc, C, groups, xt, x_own, idx16a, idx16b,
              mneg, w1, as1, ad1, b1, w2, sc2, out_d, h2t_slice, h2t_full,
              cfg)
    return nc


def _gather_select(nc, tc, C, table_d, srcp, xs, tag, groups=None,
                   group_c0=None, own=None, pool_cols=()):
    """xs[p, c] = table[srcp[p, c]] via per-column indirect row gathers.

    Every node's rank-0 edge is its self-loop, so the first column of each
    block holds the node's own value: filled with a strided DVE copy instead
    of 128 DMA descriptors.  Only pool_groups' columns are gathered here;
    the rest are produced by _pe_gather."""
    skip = set()
    if groups is not None:
        for (a, b, w) in groups:
            nb = b - a
            c0 = group_c0[a]
            skip.update(c0 + i * w for i in range(nb))
            nc.vector.tensor_copy(
                out=xs[:, c0:c0 + nb * w:w], in_=own[:, a:b])
    for c in pool_cols:
        nc.gpsimd.indirect_dma_start(
            out=xs[:, c:c + 1], out_offset=None, in_=table_d,
            in_offset=bass.IndirectOffsetOnAxis(ap=srcp[:, c:c + 1],
                                                axis=0))


def _pe_gather(nc, tc, xs, qrows, lanes, tabA, tabB, iotal, iotap, oh32,
               pe_cols):
    """xs[p, pe_cols[j]] = tab[q[p, j], lane[p, j]] via one-hot f32r matmuls.

    Per column: broadcast the 128 q-values across partitions (ones-matmul),
    build the two [128, 128] one-hot chunks with is_equal vs a partition
    iota, row-gather the [256, TABL] table into PSUM with two chained f32r
    matmuls, then select the lane with a fused is_equal*mult and reduce."""
    import contextlib
    B = 7
    ctx = contextlib.ExitStack()
    with ctx:
        psb = ctx.enter_context(tc.tile_pool(name="psb", bufs=2, space="PSUM"))
        psy = ctx.enter_context(tc.tile_pool(name="psy", bufs=3, space="PSUM"))
        mk = ctx.enter_context(tc.tile_pool(name="mk", bufs=4))
        nb = len(pe_cols)
        for j0 in range(0, nb, B):
            cols = pe_cols[j0:j0 + B]
            m = len(cols)
            pb = psb.tile([P, B * P], F32, tag="pb")
            for i in range(m):
                j = j0 + i
                jb, jr = j // 32, j % 32
                nc.tensor.matmul(pb[:, i * P:(i + 1) * P],
                                 lhsT=oh32[:, jr * P:(jr + 1) * P],
                                 rhs=qrows[:, jb * P:(jb + 1) * P],
                                 start=True, stop=True)
            mask = mk.tile([P, 2 * B * P], F32R, tag="mk")
            nc.vector.tensor_scalar(out=mask[:, 0:m * P], in0=pb[:, 0:m * P],
                                    scalar1=iotap[:, 0:1], scalar2=None,
                                    op0=AT.is_equal)
            nc.vector.tensor_scalar(out=mask[:, B * P:B * P + m * P],
                                    in0=pb[:, 0:m * P],
                                    scalar1=iotap[:, 1:2], scalar2=None,
                                    op0=AT.is_equal)
            for i in range(m):
                j = j0 + i
                c = cols[i]
                py = psy.tile([P, TABL], F32, tag="py")
                nc.tensor.matmul(py[:], lhsT=mask[:, i * P:(i + 1) * P],
                                 rhs=tabA[:], start=True, stop=False)
                nc.tensor.matmul(py[:],
                                 lhsT=mask[:, B * P + i * P:B * P + (i + 1) * P],
                                 rhs=tabB[:], start=False, stop=True)
                tmp = mk.tile([P, TABL], F16, tag="tmp")
                nc.vector.scalar_tensor_tensor(
                    out=tmp[:], in0=iotal[:], scalar=lanes[:, j:j + 1],
                    in1=py[:], op0=AT.is_equal, op1=AT.mult)
                dum = mk.tile([P, TABL], F16, tag="dum")
                nc.scalar.activation(out=dum[:], in_=tmp[:], func=AF.Copy,
                                     accum_out=xs[:, c:c + 1])


def _body(nc, tc, C, groups, xt_d, x_own_d, srcpa_d, srcpb_d,
          mneg_d, w1_d, as1_d, ad1_d, b1_d, w2_d, sc2_d,
          out_d, h2t_slice, h2t_full, cfg):
    import contextlib
    ctx = contextlib.ExitStack()
    H = 8
    with ctx:
        const = ctx.enter_context(tc.tile_pool(name="const", bufs=1))
        group_c0 = {}
        _c = 0
        for (ga, gb, gw) in groups:
            group_c0[ga] = _c
            _c += gw * (gb - ga)

        # ---- persistent loads
        mneg = const.tile([P, C], F32)
        nc.sync.dma_start(mneg[:], mneg_d[:])
        srcpa = const.tile([P, C], mybir.dt.int32)
        nc.sync.dma_start(srcpa[:], srcpa_d[:])
        x_own = const.tile([P, NBLK], F32)
        nc.sync.dma_start(x_own[:], x_own_d[:])

        # ---- params: one row, then broadcast via ones-matmul
        # 0:128 w1 | 128:256 as1 | 256:384 ad1 | 384:512 b1 | 512:640 w2
        # 640:648 sc2 (att_src2, att_dst2, b2, w2sum | iota4)
        # 648:656 s1 | 656:664 d1
        prow = const.tile([1, 664], F32)
        nc.sync.dma_start(prow[:, 0:128], w1_d[:])
        nc.sync.dma_start(prow[:, 128:256], as1_d[:])
        nc.sync.dma_start(prow[:, 256:384], ad1_d[:])
        nc.sync.dma_start(prow[:, 384:512], b1_d[:])
        nc.sync.dma_start(prow[:, 512:640], w2_d[:])
        nc.sync.dma_start(prow[:, 640:648], sc2_d[:])
        tmp = const.tile([1, 256], F32)
        nc.vector.tensor_tensor(out=tmp[:, 0:128], in0=prow[:, 0:128],
                                in1=prow[:, 128:256], op=AT.mult)
        nc.vector.tensor_tensor(out=tmp[:, 128:256], in0=prow[:, 0:128],
                                in1=prow[:, 256:384], op=AT.mult)
        nc.vector.tensor_reduce(out=prow[:, 648:664],
                                in_=tmp[:].rearrange("a (h c) -> a h c", c=16),
                                axis=mybir.AxisListType.X, op=AT.add)
        nc.vector.tensor_reduce(out=prow[:, 643:644], in_=prow[:, 512:640],
                                axis=mybir.AxisListType.X, op=AT.add)

        ones = const.tile([1, P], F32)
        nc.vector.memset(ones[:], 1.0)
        # funnel prow through one DVE copy so the matmul (whose load-weights
        # encoding has a tight sem-wait budget) depends on a single producer
        prow2 = const.tile([1, 664], F32)
        nc.vector.tensor_copy(out=prow2[:], in_=prow[:])
        pc = const.tile([P, 664], F32)
        with tc.tile_pool(name="psum", bufs=2, space="PSUM") as psum:
            for lo, hi in ((0, 512), (512, 664)):
                pcast = psum.tile([P, 512], F32, tag="pcast")
                nc.tensor.matmul(pcast[:, :hi - lo], lhsT=ones[:],
                                 rhs=prow2[:, lo:hi], start=True, stop=True)
                nc.vector.tensor_copy(out=pc[:, lo:hi],
                                      in_=pcast[:, :hi - lo])
        W1t = pc[:, 0:128]
        B1t = pc[:, 384:512]
        W2t = pc[:, 512:640]
        s2c = pc[:, 640:641]
        d2c = pc[:, 641:642]
        b2c = pc[:, 642:643]
        w2sum = pc[:, 643:644]
        iota4 = pc[:, 644:648]
        s1c = pc[:, 648:656]
        d1c = pc[:, 656:664]

        # ---- PE-gather constants
        n_pe = max(len(cfg["pe_cols"]), 1)
        qr1 = const.tile([32, cfg["nb_pe"] * P], F16)
        nc.sync.dma_start(qr1[:], cfg["q1r_d"][:])
        ln1 = const.tile([P, n_pe], F32)
        nc.sync.dma_start(ln1[:], cfg["lane1_d"][:])
        qr2 = const.tile([32, cfg["nb_pe"] * P], F16)
        nc.sync.dma_start(qr2[:], cfg["q2r_d"][:])
        ln2 = const.tile([P, n_pe], F32)
        nc.sync.dma_start(ln2[:], cfg["lane2_d"][:])
        oh32 = const.tile([32, 32 * P], F16)
        nc.sync.dma_start(oh32[:], cfg["oh32_d"][:])
        iotal = const.tile([P, TABL], F16)
        nc.sync.dma_start(iotal[:], cfg["iotal_d"][:])
        iotap = const.tile([P, 2], F32)
        nc.sync.dma_start(iotap[:], cfg["iotap_d"][:])
        tabA = const.tile([P, TABL], F32R)
        nc.sync.dma_start(tabA[:], cfg["xtab"][0:P, :])
        tabB = const.tile([P, TABL], F32R)
        nc.sync.dma_start(tabB[:], cfg["xtab"][P:256, :])

        # ---- gather x[src] (layer 1)
        xs = const.tile([P, C], F32)
        _gather_select(nc, tc, C, xt_d[:], srcpa, xs, "a",
                       groups=groups, group_c0=group_c0, own=x_own,
                       pool_cols=cfg["pool_cols"])
        if cfg["pe_cols"]:
            _pe_gather(nc, tc, xs, qr1, ln1, tabA, tabB, iotal, iotap,
                       oh32, cfg["pe_cols"])

        # adst[p, b, h] = x_own[p, b] * d1[h]
        adst = const.tile([P, NBLK * H], F32)
        nc.vector.tensor_tensor(
            out=adst[:].rearrange("p (b h) -> p b h", h=H),
            in0=x_own[:].rearrange("p b -> p b ()").to_broadcast([P, NBLK, H]),
            in1=d1c.rearrange("p h -> p () h").to_broadcast([P, NBLK, H]),
            op=AT.mult)

        denom = const.tile([P, NBLK * H], F32)
        zt = const.tile([P, NBLK * H], F32)

        # ---- layer-1 main, one run per block-group
        with tc.tile_pool(name="work", bufs=2) as work:
            _layer1_main(nc, C, groups, group_c0, work, xs, mneg, adst, s1c,
                         denom, zt)

        # ---- layer-1 epilogue -> h2_own [P, NBLK]
        r = const.tile([P, NBLK * H], F32)
        nc.vector.tensor_scalar(out=r[:], in0=denom[:], scalar1=float(EPS),
                                scalar2=None, op0=AT.add)
        nc.vector.reciprocal(out=r[:], in_=r[:])
        nc.vector.tensor_tensor(out=r[:], in0=r[:], in1=zt[:], op=AT.mult)

        h2_own = const.tile([P, NBLK], F32)
        with tc.tile_pool(name="ep", bufs=2) as ep:
            _epilogue(nc, ep, r, W1t, B1t, W2t, h2_own)
        nc.vector.tensor_scalar(out=h2_own[:], in0=h2_own[:], scalar1=w2sum,
                                scalar2=None, op0=AT.subtract)
        pe2 = {"qr": qr2, "ln": ln2, "iotal": iotal, "iotap": iotap,
               "oh32": oh32, "pe_cols": cfg["pe_cols"],
               "pool_cols": cfg["pool_cols"]}
        _rest(nc, tc, C, groups, group_c0, const, mneg, h2_own, srcpa,
              srcpb_d, s2c, d2c, b2c, out_d, h2t_slice, h2t_full, pe2)


def _layer1_main(nc, C, groups, group_c0, work, xs, mneg, adst, s1c,
                 denom, zt):
        H = 8
        for (a, b, w) in groups:
            nb = b - a
            c0 = group_c0[a]
            V = nb * H * w
            xs_g = xs[:, c0:c0 + nb * w].rearrange("p (n w) -> p n () w", w=w)
            mn_g = mneg[:, c0:c0 + nb * w].rearrange("p (n w) -> p n () w", w=w)
            ad_g = adst[:, a * H:b * H].rearrange("p (n h) -> p n h ()", h=H)
            s1_g = s1c.rearrange("p h -> p () h ()")

            u = work.tile([P, V], F32, tag="u")
            u4 = u[:].rearrange("p (n h w) -> p n h w", h=H, w=w)
            nc.vector.tensor_tensor(out=u4, in0=xs_g.to_broadcast([P, nb, H, w]),
                                    in1=s1_g.to_broadcast([P, nb, H, w]), op=AT.mult)
            u2 = work.tile([P, V], F32, tag="u2")
            u24 = u2[:].rearrange("p (n h w) -> p n h w", h=H, w=w)
            nc.vector.tensor_tensor(out=u24, in0=u4,
                                    in1=ad_g.to_broadcast([P, nb, H, w]), op=AT.add)
            nc.vector.tensor_tensor(out=u4, in0=u24,
                                    in1=mn_g.to_broadcast([P, nb, H, w]), op=AT.add)
            # leaky relu: max(0.2*v, v), then exp
            if USE_ACT_LRELU:
                nc.scalar.activation(out=u24, in_=u4, func=AF.Lrelu,
                                     alpha=NEG_SLOPE)
            else:
                nc.vector.scalar_tensor_tensor(out=u24, in0=u4, scalar=NEG_SLOPE,
                                               in1=u4, op0=AT.mult, op1=AT.max)
            ex = work.tile([P, V], F32, tag="ex")
            ex4 = ex[:].rearrange("p (n h w) -> p n h w", h=H, w=w)
            nc.scalar.activation(out=ex4, in_=u24, func=AF.Exp)
            nc.vector.tensor_reduce(
                out=denom[:, a * H:b * H].rearrange("p (n h) -> p n h", h=H),
                in_=ex4, axis=mybir.AxisListType.X, op=AT.add)
            nc.vector.tensor_tensor(out=u4, in0=ex4,
                                    in1=xs_g.to_broadcast([P, nb, H, w]), op=AT.mult)
            nc.vector.tensor_reduce(
                out=zt[:, a * H:b * H].rearrange("p (n h) -> p n h", h=H),
                in_=u4, axis=mybir.AxisListType.X, op=AT.add)

def _epilogue(nc, ep, r, W1t, B1t, W2t, h2_own):
        H = 8
        EPB = 14
        for a in range(0, NBLK, EPB):
            b = min(a + EPB, NBLK)
            nb = b - a
            V = nb * 128
            v = ep.tile([P, EPB * 128], F32, tag="v")
            v4 = v[:, :V].rearrange("p (n h c) -> p n h c", h=H, c=16)
            r_g = r[:, a * H:b * H].rearrange("p (n h) -> p n h ()", h=H)
            w1_g = W1t.rearrange("p (h c) -> p () h c", c=16)
            b1_g = B1t.rearrange("p (h c) -> p () h c", c=16)
            nc.vector.tensor_tensor(out=v4, in0=r_g.to_broadcast([P, nb, H, 16]),
                                    in1=w1_g.to_broadcast([P, nb, H, 16]),
                                    op=AT.mult)
            v2 = ep.tile([P, EPB * 128], F32, tag="v2")
            nc.vector.tensor_tensor(
                out=v2[:, :V].rearrange("p (n h c) -> p n h c", h=H, c=16),
                in0=v4, in1=b1_g.to_broadcast([P, nb, H, 16]), op=AT.add)
            # h1' = max(v,0) + min(exp(v),1);  elu(v) = h1' - 1
            ev = ep.tile([P, EPB * 128], F32, tag="ev")
            nc.scalar.activation(out=ev[:, :V], in_=v2[:, :V], func=AF.Exp)
            nc.vector.tensor_scalar(out=ev[:, :V], in0=ev[:, :V], scalar1=1.0,
                                    scalar2=None, op0=AT.min)
            nc.vector.tensor_scalar(out=v2[:, :V], in0=v2[:, :V], scalar1=0.0,
                                    scalar2=None, op0=AT.max)
            nc.vector.tensor_tensor(out=v2[:, :V], in0=v2[:, :V], in1=ev[:, :V],
                                    op=AT.add)
            # h2 = sum h1'*W2 - W2sum  (the elu -1 folded into W2sum)
            w2_g = W2t.rearrange("p (h c) -> p () (h c)", c=16)
            nc.vector.tensor_tensor(
                out=v4, in0=v2[:, :V].rearrange("p (n f) -> p n f", f=128),
                in1=w2_g.to_broadcast([P, nb, 128]), op=AT.mult)
            nc.vector.tensor_reduce(
                out=h2_own[:, a:b], in_=v4.rearrange("p n h c -> p n (h c)"),
                axis=mybir.AxisListType.X, op=AT.add)
def _rest(nc, tc, C, groups, group_c0, const, mneg, h2_own, srcpa,
          srcpb_d, s2c, d2c, b2c, out_d, h2t_slice, h2t_full, pe2):
        # ---- store own h2 slice, AllGather the table
        nc.sync.dma_start(
            h2t_slice[:].rearrange("q l -> (q l)").rearrange(
                "(b p) -> p b", p=P),
            h2_own[:])
        nc.gpsimd.collective_compute(
            "AllGather", AT.bypass,
            replica_groups=[list(range(NCORES))],
            ins=[h2t_slice[:]], outs=[h2t_full[:]])

        # ---- layer-2 PE tables from the gathered h2 (rounded to f32r)
        h2tab_f = const.tile([P, 2 * TABL], F32)
        h2t2d = h2t_full[:].rearrange("(q l) one -> q (l one)", q=256)
        nc.sync.dma_start(h2tab_f[:, 0:TABL], h2t2d[0:P, :])
        nc.sync.dma_start(h2tab_f[:, TABL:2 * TABL], h2t2d[P:256, :])
        h2tabA = const.tile([P, TABL], F32R)
        nc.vector.tensor_copy(out=h2tabA[:], in_=h2tab_f[:, 0:TABL])
        h2tabB = const.tile([P, TABL], F32R)
        nc.vector.tensor_copy(out=h2tabB[:], in_=h2tab_f[:, TABL:2 * TABL])

        # ---- layer 2: reuse the srcpa tile's SBUF for srcpb
        nc.sync.dma_start(srcpa[:], srcpb_d[:])
        h2s = const.tile([P, C], F32)
        _gather_select(nc, tc, C, h2t_full[:], srcpa, h2s, "b",
                       groups=groups, group_c0=group_c0, own=h2_own,
                       pool_cols=pe2["pool_cols"])
        if pe2["pe_cols"]:
            _pe_gather(nc, tc, h2s, pe2["qr"], pe2["ln"], h2tabA, h2tabB,
                       pe2["iotal"], pe2["iotap"], pe2["oh32"],
                       pe2["pe_cols"])

        adst2 = const.tile([P, NBLK], F32)
        nc.vector.tensor_scalar(out=adst2[:], in0=h2_own[:], scalar1=d2c,
                                scalar2=None, op0=AT.mult)

        den2 = const.tile([P, NBLK], F32)
        z2 = const.tile([P, NBLK], F32)
        with tc.tile_pool(name="work2", bufs=2) as work:
            _layer2_main(nc, groups, group_c0, work, h2s, mneg, adst2, s2c,
                         den2, z2)
        _output(nc, den2, z2, b2c, out_d)


def _layer2_main(nc, groups, group_c0, work, h2s, mneg, adst2, s2c, den2, z2):
        for (a, b, w) in groups:
            nb = b - a
            c0 = group_c0[a]
            V = nb * w
            sl = slice(c0, c0 + V)
            h2s_g = h2s[:, sl].rearrange("p (n w) -> p n w", w=w)
            u = work.tile([P, V], F32, tag="u")
            u3 = u[:].rearrange("p (n w) -> p n w", w=w)
            nc.vector.scalar_tensor_tensor(
                out=u3, in0=h2s_g, scalar=s2c,
                in1=adst2[:, a:b].rearrange("p n -> p n ()").to_broadcast(
                    [P, nb, w]),
                op0=AT.mult, op1=AT.add)
            u2 = work.tile([P, V], F32, tag="u2")
            u23 = u2[:].rearrange("p (n w) -> p n w", w=w)
            nc.vector.tensor_tensor(
                out=u23, in0=u3,
                in1=mneg[:, sl].rearrange("p (n w) -> p n w", w=w), op=AT.add)
            if USE_ACT_LRELU:
                nc.scalar.activation(out=u3, in_=u23, func=AF.Lrelu,
                                     alpha=NEG_SLOPE)
            else:
                nc.vector.scalar_tensor_tensor(out=u3, in0=u23, scalar=NEG_SLOPE,
                                               in1=u23, op0=AT.mult, op1=AT.max)
            nc.scalar.activation(out=u23, in_=u3, func=AF.Exp)
            nc.vector.tensor_reduce(out=den2[:, a:b], in_=u23,
                                    axis=mybir.AxisListType.X, op=AT.add)
            nc.vector.tensor_tensor(out=u3, in0=u23, in1=h2s_g, op=AT.mult)
            nc.vector.tensor_reduce(out=z2[:, a:b], in_=u3,
                                    axis=mybir.AxisListType.X, op=AT.add)


def _output(nc, den2, z2, b2c, out_d):
        # ---- output
        nc.vector.tensor_scalar(out=den2[:], in0=den2[:], scalar1=float(EPS),
                                scalar2=None, op0=AT.add)
        nc.vector.reciprocal(out=den2[:], in_=den2[:])
        nc.vector.tensor_tensor(out=z2[:], in0=z2[:], in1=den2[:], op=AT.mult)
        nc.vector.tensor_scalar(out=z2[:], in0=z2[:], scalar1=b2c,
                                scalar2=None, op0=AT.add)
        nc.sync.dma_start(out_d[:], z2[:])


# ---------------------------------------------------------------------------

def kernel(**inputs):
    edge_index = np.asarray(inputs["edge_index"])
    prep = _prep(edge_index)
    C, groups, perms = prep["C"], prep["groups"], prep["perms"]

    x = np.asarray(inputs["x"], dtype=np.float32).reshape(-1)   # [N]
    xt = np.ascontiguousarray(x.reshape(N, 1))

    flat = lambda a: np.ascontiguousarray(
        np.asarray(a, dtype=np.float32).reshape(1, -1))
    w1 = flat(inputs["W1"]); as1 = flat(inputs["att_src1"])
    ad1 = flat(inputs["att_dst1"]); b1 = flat(inputs["b1"])
    w2 = flat(inputs["W2"])
    sc2 = np.zeros((1, 8), np.float32)
    sc2[0, 0] = np.asarray(inputs["att_src2"]).reshape(-1)[0]
    sc2[0, 1] = np.asarray(inputs["att_dst2"]).reshape(-1)[0]
    sc2[0, 2] = np.asarray(inputs["b2"]).reshape(-1)[0]
    sc2[0, 4:8] = [0.0, 1.0, 2.0, 3.0]

    nc = _build(prep)

    xtab = np.zeros(256 * TABL, np.float32)
    xtab[:N] = x
    xtab = np.ascontiguousarray(xtab.reshape(256, TABL))
    iotal = np.broadcast_to(np.arange(TABL, dtype=np.float16), (P, TABL))
    iotal = np.ascontiguousarray(iotal)
    iotap = np.stack([np.arange(P, dtype=np.float32),
                      np.arange(P, dtype=np.float32) + P], axis=1)
    iotap = np.ascontiguousarray(iotap)
    oh32 = np.zeros((32, 32, P), np.float16)
    for j in range(32):
        oh32[j, j, :] = 1.0
    oh32 = np.ascontiguousarray(oh32.reshape(32, 32 * P))

    in_maps = []
    for k in range(NCORES):
        xk = x[k * NPC:(k + 1) * NPC][perms[k]]
        xk = np.concatenate([xk, np.zeros(ROWS - NPC, np.float32)])
        x_own = np.ascontiguousarray(xk.reshape(NBLK, P).T)
        in_maps.append({
            "xt": xt, "x_own": x_own,
            "srcpa": prep["srcpa"][k], "srcpb": prep["srcpb"][k],
            "mneg": prep["maskneg"][k],
            "w1": w1, "as1": as1, "ad1": ad1, "b1": b1, "w2": w2, "sc2": sc2,
            "xtab": xtab,
            "q1rows": np.ascontiguousarray(prep["q1rows"][k]),
            "lane1": np.ascontiguousarray(prep["lane1"][k]),
            "q2rows": np.ascontiguousarray(prep["q2rows"][k]),
            "lane2": np.ascontiguousarray(prep["lane2"][k]),
            "iotal": iotal, "iotap": iotap, "oh32": oh32,
        })

    res = run_bass_kernel_spmd(nc, in_maps, core_ids=list(range(NCORES)))

    out = np.zeros((N, 1), np.float32)
    for k in range(NCORES):
        o = res.results[k]["out"]                    # [P, NBLK]
        flat_o = o.T.reshape(-1)[:NPC]
        out[k * NPC:(k + 1) * NPC, 0][perms[k]] = flat_o
    return out

